# revision 12
# baseline (speedup 1.0000x reference)
"""Trainium2 Bass kernel for nn_CompetitiveInhibitorLayer.

For hidden_states [4,4096,2048] f32, input_ids [4,4096] i64, inhibitor_weight [1] f32:

    trigger = decay-max scan over (ids in {5,13,42,99}):  m[t] = max(hit[t], m[t-1]*0.95)
    payload = 1.0 iff sum_h h^2 < 5e-25   (== normalized self-sim < 0.5)
    mask    = 1 - max(trigger, payload) * w
    out     = hidden_states * mask[..., None]
    returns (out, mask)

Sharding: 8 cores = (batch b in 0..3) x (sequence half in 0..1); each core
streams its [2048, 2048] hidden slice (16.8MB in + 16.8MB out -> HBM-line
bound, ~94us/core). The scan needs the carry from the first half, so each
core's ids input is [prev_half | own_half] (zeros prefix for first-half
cores) and the scan runs over the full 4096 on-device via the DVE
tensor_tensor_scan (bit-exact vs the reference recurrence).

Engine plan per core:
  Sync   (HWDGE): ids/dec/ident loads, 8 x 2MB hidden loads, msb relayout,
                  16 x 1MB scaled-tile stores + mask store (gated per tile).
  Scalar (ACT):   hit->hitrow SBUF relayout DMA, then 16 x Square+accumulate
                  (per-token sum of squares).
  Vector (DVE):   trigger-hit compare/max chain, the serial decay scan,
                  batched factor math, 16 in-place tile scalings (2x mode).
  Tensor (PE):    one 16x128 transpose putting the trigger mask into
                  per-tile [128,1] columns (PSUM).
"""

import sys

import numpy as np

if "/opt/trn_rl_repo" not in sys.path:
    sys.path.insert(0, "/opt/trn_rl_repo")

B, S, H = 4, 4096, 2048
NCORES = 8
SLOC = S // 2  # tokens per core
P = 128
NT = SLOC // P  # 16 tiles per core
NG = NT // 2    # 8 x 2MB input DMA groups
DECAY = 0.95
THR = 5e-25  # payload: ss < 0.5 * (1e-12)^2
TRIGGERS = (5.0, 13.0, 42.0, 99.0)

# test harness knobs (the graded harness calls kernel(**inputs) only)
TRACE = False
LAST_RESULT = None


def _build(w: float):
    import concourse.bass as bass
    import concourse.mybir as mybir
    from contextlib import ExitStack

    f32 = mybir.dt.float32
    Alu = mybir.AluOpType
    Act = mybir.ActivationFunctionType

    nc = bass.Bass()
    hs = nc.declare_dram_parameter("hs", [SLOC, H], f32, isOutput=False)
    idscat = nc.declare_dram_parameter("idscat", [S], f32, isOutput=False)
    dec = nc.declare_dram_parameter("dec", [S], f32, isOutput=False)
    ident = nc.declare_dram_parameter("ident", [NT, NT], f32, isOutput=False)
    out = nc.declare_dram_parameter("out", [SLOC, H], f32, isOutput=True)
    # msk holds fb [128,16] flattened partition-major: msk[16p+i] = factor(token 128i+p)
    msk = nc.declare_dram_parameter("msk", [SLOC], f32, isOutput=True)

    ctx = ExitStack()
    sb = lambda name, shape: ctx.enter_context(nc.sbuf_tensor(name, shape, f32))
    with ctx:
        ids_sb = sb("ids_sb", [P, S // P])   # [128, 32], s = 32p + f
        tmp_sb = sb("tmp_sb", [P, S // P])
        hit_sb = sb("hit_sb", [P, S // P])
        hitrow = sb("hitrow", [1, S])
        decrow = sb("decrow", [1, S])
        mrow = sb("mrow", [1, S])
        msb = sb("msb", [NT, P])             # [16, 128]: msb[t, p] = m_own[128t + p]
        idn = sb("idn", [NT, NT])            # 16x16 identity (transpose helper)
        ss = sb("ss", [P, NT])               # per-tile sum-of-squares column
        fb = sb("fb", [P, NT])               # per-tile factor column
        ps = sb("ps", [P, 4])
        # 8 groups of 2 tiles: hsb2[g][:, c*H:(c+1)*H] = hidden rows 256g+128c+p
        hsb2 = [sb(f"hsb2_{g}", [P, 2 * H]) for g in range(NG)]
        sqp = ctx.enter_context(nc.psum_tensor("sqp", [P, H], f32))
        mtp = ctx.enter_context(nc.psum_tensor("mtp", [P, NT], f32))

        s_ids = ctx.enter_context(nc.semaphore("s_ids"))
        s_dec = ctx.enter_context(nc.semaphore("s_dec"))
        s_in = [ctx.enter_context(nc.semaphore(f"s_in{g}")) for g in range(NG)]
        s_hit = ctx.enter_context(nc.semaphore("s_hit"))
        s_hr = ctx.enter_context(nc.semaphore("s_hr"))
        s_scan = ctx.enter_context(nc.semaphore("s_scan"))
        s_msb = ctx.enter_context(nc.semaphore("s_msb"))
        s_mt = ctx.enter_context(nc.semaphore("s_mt"))
        s_sq = ctx.enter_context(nc.semaphore("s_sq"))
        s_scale = ctx.enter_context(nc.semaphore("s_scale"))
        s_out = ctx.enter_context(nc.semaphore("s_out"))

        block = ctx.enter_context(nc.Block())

        @block.sync
        def _(sync):
            sync.dma_start(
                out=ids_sb[:], in_=idscat[:].rearrange("(p f) -> p f", f=S // P)
            ).then_inc(s_ids, 16)
            sync.dma_start(out=decrow[:], in_=dec[None, :]).then_inc(s_dec, 16)
            sync.dma_start(out=idn[:], in_=ident[:]).then_inc(s_dec, 16)
            for g in range(NG):
                sync.dma_start(
                    out=hsb2[g][:].rearrange("p (c h) -> p c h", c=2),
                    in_=hs[2 * P * g : 2 * P * (g + 1), :].rearrange(
                        "(c p) h -> p c h", p=P
                    ),
                ).then_inc(s_in[g], 16)
            # own half of the scan -> msb [16,128] (SBUF->SBUF relayout)
            sync.wait_ge(s_scan, 1)
            sync.dma_start(out=msb[:], in_=mrow[0:1, SLOC:]).then_inc(s_msb, 16)
            # scaled tiles out, gated on the in-place scale of each tile
            for i in range(NT):
                sync.wait_ge(s_scale, i + 1)
                sync.dma_start(
                    out=out[i * P : (i + 1) * P, :],
                    in_=hsb2[i // 2][:, (i % 2) * H : (i % 2 + 1) * H],
                ).then_inc(s_out, 16)
            sync.dma_start(
                out=msk[:].rearrange("(p f) -> p f", f=NT), in_=fb[:]
            ).then_inc(s_out, 16)
            sync.wait_ge(s_out, 16 * (NT + 1))

        @block.vector
        def _(vector):
            # hit = (ids==5)|(ids==13)|(ids==42)|(ids==99), in [128,32] layout
            vector.wait_ge(s_ids, 16)
            vector.tensor_scalar(hit_sb[:], ids_sb[:], TRIGGERS[0], None, Alu.is_equal)
            last = None
            for t in TRIGGERS[1:]:
                vector.drain()
                vector.tensor_scalar(tmp_sb[:], ids_sb[:], t, None, Alu.is_equal)
                vector.drain()
                last = vector.tensor_tensor(hit_sb[:], hit_sb[:], tmp_sb[:], Alu.max)
            last.then_inc(s_hit, 1)
            # serial decay scan over the full row: state = max(decay*state, hit[t])
            vector.wait_ge(s_hr, 16)
            vector.wait_ge(s_dec, 32)
            vector.tensor_tensor_scan(
                mrow[0:1, :], decrow[0:1, :], hitrow[0:1, :], 0.0, Alu.mult, Alu.max
            ).then_inc(s_scan, 1)
            # factor math batched in groups of 4 tiles, then in-place scaling
            vector.wait_ge(s_mt, 1)  # PE transpose of trigger mask done
            for j in range(NT // 4):
                c4 = slice(4 * j, 4 * j + 4)
                vector.wait_ge(s_sq, 4 * j + 4)
                vector.drain()
                # ps = (ss < THR) max trigger   (payload-or-trigger, fused)
                vector.scalar_tensor_tensor(
                    ps[:], ss[:, c4], THR, mtp[:, c4], Alu.is_lt, Alu.max
                )
                vector.drain()
                vector.tensor_scalar(fb[:, c4], ps[:], -w, 1.0, Alu.mult, Alu.add)
                vector.drain()
                for k in range(4):
                    i = 4 * j + k
                    vector.tensor_scalar(
                        hsb2[i // 2][:, (i % 2) * H : (i % 2 + 1) * H],
                        hsb2[i // 2][:, (i % 2) * H : (i % 2 + 1) * H],
                        fb[:, i : i + 1],
                        None,
                        Alu.mult,
                    ).then_inc(s_scale, 1)

        @block.scalar
        def _(scalar):
            # hit [128,32] -> hitrow [1,4096] (SBUF->SBUF relayout on ACT HWDGE)
            scalar.wait_ge(s_hit, 1)
            scalar.dma_start(out=hitrow[:], in_=hit_sb[:]).then_inc(s_hr, 16)
            for i in range(NT):
                scalar.wait_ge(s_in[i // 2], 16)
                if i:
                    scalar.drain()
                scalar.activation(
                    sqp[:],
                    hsb2[i // 2][:, (i % 2) * H : (i % 2 + 1) * H],
                    Act.Square,
                    accum_out=ss[:, i : i + 1],
                ).then_inc(s_sq, 1)

        @block.tensor
        def _(tensor):
            tensor.wait_ge(s_msb, 16)
            tensor.wait_ge(s_dec, 32)  # identity loaded
            tensor.transpose(mtp[:], msb[:], idn[:]).then_inc(s_mt, 1)

    return nc


def _in_maps(hidden_states, input_ids):
    ids_f = np.asarray(input_ids).astype(np.float32)  # values < 1000: exact
    hsf = np.asarray(hidden_states, dtype=np.float32)
    decv = np.full(S, DECAY, dtype=np.float32)
    identv = np.eye(NT, dtype=np.float32)
    maps = []
    for c in range(NCORES):
        b, h = divmod(c, 2)
        off = h * SLOC
        prev = ids_f[b, :SLOC] if h == 1 else np.zeros(SLOC, dtype=np.float32)
        idscat = np.concatenate([prev, ids_f[b, off : off + SLOC]])
        maps.append(
            {
                "hs": np.ascontiguousarray(hsf[b, off : off + SLOC, :]),
                "idscat": np.ascontiguousarray(idscat),
                "dec": decv,
                "ident": identv,
            }
        )
    return maps


def kernel(hidden_states, input_ids, inhibitor_weight):
    global LAST_RESULT
    from concourse.bass_utils import run_bass_kernel_spmd

    w = float(np.asarray(inhibitor_weight).reshape(-1)[0])
    nc = _build(w)
    maps = _in_maps(hidden_states, input_ids)
    res = run_bass_kernel_spmd(nc, maps, core_ids=list(range(NCORES)), trace=TRACE)
    LAST_RESULT = res

    full_out = np.empty((B, S, H), dtype=np.float32)
    full_mask = np.empty((B, S), dtype=np.float32)
    for c in range(NCORES):
        b, h = divmod(c, 2)
        off = h * SLOC
        full_out[b, off : off + SLOC, :] = res.results[c]["out"]
        # msk[16p+i] = factor(token 128i+p) -> unpermute to token order
        full_mask[b, off : off + SLOC] = (
            res.results[c]["msk"].reshape(P, NT).T.reshape(-1)
        )
    return full_out, full_mask


# revision 14
# speedup vs baseline: 1.0652x; 1.0652x over previous
"""Trainium2 Bass kernel for nn_CompetitiveInhibitorLayer.

For hidden_states [4,4096,2048] f32, input_ids [4,4096] i64, inhibitor_weight [1] f32:

    trigger = decay-max scan over (ids in {5,13,42,99}):  m[t] = max(hit[t], m[t-1]*0.95)
    payload = 1.0 iff sum_h h^2 < 5e-25   (== normalized self-sim < 0.5)
    mask    = 1 - max(trigger, payload) * w
    out     = hidden_states * mask[..., None]
    returns (out, mask)

Sharding: 8 cores = (batch b in 0..3) x (sequence half in 0..1); each core
streams its [2048, 2048] hidden slice (16.8MB in + 16.8MB out -> HBM-line
bound, ~94us/core). The scan needs the carry from the first half, so each
core's ids input is [prev_half | own_half] (zeros prefix for first-half
cores) and the scan runs over the full 4096 on-device via the DVE
tensor_tensor_scan (bit-exact vs the reference recurrence).

Engine plan per core:
  Sync   (HWDGE): ids/dec/ident loads, 8 x 2MB hidden loads, msb relayout,
                  16 x 1MB scaled-tile stores + mask store (gated per tile).
  Scalar (ACT):   hit->hitrow SBUF relayout DMA, then 16 x Square+accumulate
                  (per-token sum of squares).
  Vector (DVE):   trigger-hit compare/max chain, the serial decay scan,
                  batched factor math, 16 in-place tile scalings (2x mode).
  Tensor (PE):    one 16x128 transpose putting the trigger mask into
                  per-tile [128,1] columns (PSUM).
"""

import sys

import numpy as np

if "/opt/trn_rl_repo" not in sys.path:
    sys.path.insert(0, "/opt/trn_rl_repo")

B, S, H = 4, 4096, 2048
NCORES = 8
SLOC = S // 2  # tokens per core
P = 128
NT = SLOC // P  # 16 tiles per core
NG = NT // 2    # 8 x 2MB input DMA groups
DECAY = 0.95
THR = 5e-25  # payload: ss < 0.5 * (1e-12)^2
TRIGGERS = (5.0, 13.0, 42.0, 99.0)

# test harness knobs (the graded harness calls kernel(**inputs) only)
TRACE = False
LAST_RESULT = None


def _build(w: float):
    import concourse.bass as bass
    import concourse.mybir as mybir
    from contextlib import ExitStack

    f32 = mybir.dt.float32
    Alu = mybir.AluOpType
    Act = mybir.ActivationFunctionType

    nc = bass.Bass()
    hs = nc.declare_dram_parameter("hs", [SLOC, H], f32, isOutput=False)
    idscat = nc.declare_dram_parameter("idscat", [S], f32, isOutput=False)
    dec = nc.declare_dram_parameter("dec", [S], f32, isOutput=False)
    ident = nc.declare_dram_parameter("ident", [NT, NT], f32, isOutput=False)
    out = nc.declare_dram_parameter("out", [SLOC, H], f32, isOutput=True)
    # msk holds fb [128,16] flattened partition-major: msk[16p+i] = factor(token 128i+p)
    msk = nc.declare_dram_parameter("msk", [SLOC], f32, isOutput=True)

    ctx = ExitStack()
    sb = lambda name, shape: ctx.enter_context(nc.sbuf_tensor(name, shape, f32))
    with ctx:
        ids_sb = sb("ids_sb", [P, S // P])   # [128, 32], s = 32p + f
        tmp_sb = sb("tmp_sb", [P, S // P])
        hit_sb = sb("hit_sb", [P, S // P])
        hitrow = sb("hitrow", [1, S])
        decrow = sb("decrow", [1, S])
        mrow = sb("mrow", [1, S])
        msb = sb("msb", [NT, P])             # [16, 128]: msb[t, p] = m_own[128t + p]
        idn = sb("idn", [NT, NT])            # 16x16 identity (transpose helper)
        ss = sb("ss", [P, NT])               # per-tile sum-of-squares column
        fb = sb("fb", [P, NT])               # per-tile factor column
        ps = sb("ps", [P, 4])
        # 8 groups of 2 tiles: hsb2[g][:, c*H:(c+1)*H] = hidden rows 256g+128c+p
        hsb2 = [sb(f"hsb2_{g}", [P, 2 * H]) for g in range(NG)]
        sqp = ctx.enter_context(nc.psum_tensor("sqp", [P, H], f32))
        mtp = ctx.enter_context(nc.psum_tensor("mtp", [P, NT], f32))

        s_ids = ctx.enter_context(nc.semaphore("s_ids"))
        s_dec = ctx.enter_context(nc.semaphore("s_dec"))
        s_in = [ctx.enter_context(nc.semaphore(f"s_in{g}")) for g in range(NG)]
        s_hit = ctx.enter_context(nc.semaphore("s_hit"))
        s_hr = ctx.enter_context(nc.semaphore("s_hr"))
        s_scan = ctx.enter_context(nc.semaphore("s_scan"))
        s_msb = ctx.enter_context(nc.semaphore("s_msb"))
        s_mt = ctx.enter_context(nc.semaphore("s_mt"))
        s_sq = ctx.enter_context(nc.semaphore("s_sq"))
        s_scale = ctx.enter_context(nc.semaphore("s_scale"))
        s_out = ctx.enter_context(nc.semaphore("s_out"))

        block = ctx.enter_context(nc.Block())

        @block.sync
        def _(sync):
            sync.dma_start(
                out=ids_sb[:], in_=idscat[:].rearrange("(p f) -> p f", f=S // P)
            ).then_inc(s_ids, 16)
            sync.dma_start(out=decrow[:], in_=dec[None, :]).then_inc(s_dec, 16)
            sync.dma_start(out=idn[:], in_=ident[:]).then_inc(s_dec, 16)
            for g in range(NG):
                sync.dma_start(
                    out=hsb2[g][:].rearrange("p (c h) -> p c h", c=2),
                    in_=hs[2 * P * g : 2 * P * (g + 1), :].rearrange(
                        "(c p) h -> p c h", p=P
                    ),
                ).then_inc(s_in[g], 16)
            # scaled tiles out, gated on the in-place scale of each tile
            for i in range(NT):
                sync.wait_ge(s_scale, i + 1)
                sync.dma_start(
                    out=out[i * P : (i + 1) * P, :],
                    in_=hsb2[i // 2][:, (i % 2) * H : (i % 2 + 1) * H],
                ).then_inc(s_out, 16)
            sync.dma_start(
                out=msk[:].rearrange("(p f) -> p f", f=NT), in_=fb[:]
            ).then_inc(s_out, 16)
            sync.wait_ge(s_out, 16 * (NT + 1))

        @block.vector
        def _(vector):
            # hit = (ids==5)|(ids==13)|(ids==42)|(ids==99), in [128,32] layout
            vector.wait_ge(s_ids, 16)
            vector.tensor_scalar(hit_sb[:], ids_sb[:], TRIGGERS[0], None, Alu.is_equal)
            last = None
            for t in TRIGGERS[1:]:
                vector.drain()
                vector.tensor_scalar(tmp_sb[:], ids_sb[:], t, None, Alu.is_equal)
                vector.drain()
                last = vector.tensor_tensor(hit_sb[:], hit_sb[:], tmp_sb[:], Alu.max)
            last.then_inc(s_hit, 1)
            # serial decay scan over the full row: state = max(decay*state, hit[t])
            vector.wait_ge(s_hr, 16)
            vector.wait_ge(s_dec, 32)
            vector.tensor_tensor_scan(
                mrow[0:1, :], decrow[0:1, :], hitrow[0:1, :], 0.0, Alu.mult, Alu.max
            ).then_inc(s_scan, 1)
            # factor math batched in groups of 4 tiles, then in-place scaling
            vector.wait_ge(s_mt, 1)  # PE transpose of trigger mask done
            for j in range(NT // 4):
                c4 = slice(4 * j, 4 * j + 4)
                vector.wait_ge(s_sq, 4 * j + 4)
                vector.drain()
                # ps = (ss < THR) max trigger   (payload-or-trigger, fused)
                vector.scalar_tensor_tensor(
                    ps[:], ss[:, c4], THR, mtp[:, c4], Alu.is_lt, Alu.max
                )
                vector.drain()
                vector.tensor_scalar(fb[:, c4], ps[:], -w, 1.0, Alu.mult, Alu.add)
                vector.drain()
                for k in range(4):
                    i = 4 * j + k
                    vector.tensor_scalar(
                        hsb2[i // 2][:, (i % 2) * H : (i % 2 + 1) * H],
                        hsb2[i // 2][:, (i % 2) * H : (i % 2 + 1) * H],
                        fb[:, i : i + 1],
                        None,
                        Alu.mult,
                    ).then_inc(s_scale, 1)

        @block.scalar
        def _(scalar):
            # hit [128,32] -> hitrow [1,4096] (SBUF->SBUF relayout on ACT HWDGE)
            scalar.wait_ge(s_hit, 1)
            scalar.dma_start(out=hitrow[:], in_=hit_sb[:]).then_inc(s_hr, 16)
            for i in range(NT):
                scalar.wait_ge(s_in[i // 2], 16)
                if i:
                    scalar.drain()
                scalar.activation(
                    sqp[:],
                    hsb2[i // 2][:, (i % 2) * H : (i % 2 + 1) * H],
                    Act.Square,
                    accum_out=ss[:, i : i + 1],
                ).then_inc(s_sq, 1)
                if i == 4:
                    # own half of the scan -> msb [16,128] relayout, on the
                    # otherwise-idle ACT HWDGE ring (the SP ring is busy
                    # draining the 2MB input loads and would FIFO-delay it)
                    scalar.wait_ge(s_scan, 1)
                    scalar.dma_start(out=msb[:], in_=mrow[0:1, SLOC:]).then_inc(
                        s_msb, 16
                    )

        @block.tensor
        def _(tensor):
            tensor.wait_ge(s_msb, 16)
            tensor.wait_ge(s_dec, 32)  # identity loaded
            tensor.transpose(mtp[:], msb[:], idn[:]).then_inc(s_mt, 1)

    return nc


def _in_maps(hidden_states, input_ids):
    ids_f = np.asarray(input_ids).astype(np.float32)  # values < 1000: exact
    hsf = np.asarray(hidden_states, dtype=np.float32)
    decv = np.full(S, DECAY, dtype=np.float32)
    identv = np.eye(NT, dtype=np.float32)
    maps = []
    for c in range(NCORES):
        b, h = divmod(c, 2)
        off = h * SLOC
        prev = ids_f[b, :SLOC] if h == 1 else np.zeros(SLOC, dtype=np.float32)
        idscat = np.concatenate([prev, ids_f[b, off : off + SLOC]])
        maps.append(
            {
                "hs": np.ascontiguousarray(hsf[b, off : off + SLOC, :]),
                "idscat": np.ascontiguousarray(idscat),
                "dec": decv,
                "ident": identv,
            }
        )
    return maps


def kernel(hidden_states, input_ids, inhibitor_weight):
    global LAST_RESULT
    from concourse.bass_utils import run_bass_kernel_spmd

    w = float(np.asarray(inhibitor_weight).reshape(-1)[0])
    nc = _build(w)
    maps = _in_maps(hidden_states, input_ids)
    res = run_bass_kernel_spmd(nc, maps, core_ids=list(range(NCORES)), trace=TRACE)
    LAST_RESULT = res

    full_out = np.empty((B, S, H), dtype=np.float32)
    full_mask = np.empty((B, S), dtype=np.float32)
    for c in range(NCORES):
        b, h = divmod(c, 2)
        off = h * SLOC
        full_out[b, off : off + SLOC, :] = res.results[c]["out"]
        # msk[16p+i] = factor(token 128i+p) -> unpermute to token order
        full_mask[b, off : off + SLOC] = (
            res.results[c]["msk"].reshape(P, NT).T.reshape(-1)
        )
    return full_out, full_mask
